# revision 9
# baseline (speedup 1.0000x reference)
"""Causal Gaussian-kernel self-attention on 8 TRN2 NeuronCores (v3, bf16).

Reference computation (per batch b):
    qkv = x @ W_attn + b_attn ; q,k,v heads of 64 dims
    scores = exp(-(|q|^2 + |k|^2 - 2 q.k) / (2*sqrt(64))), causal-masked, NO softmax
    y = scores @ v ; out = y @ W_proj + b_proj

Sharding: core c -> batch b = c//2, head-group g = c%2 (8 heads each).
Per core the score factors:  exp(q.k/8) * exp(-|q|^2/16) * exp(-|k|^2/16)
  - exp(-|k|^2/16) folded into v (per-key scale, in-place per pair slice)
  - exp(-|q|^2/16) folded into the y^T PSUM->SBUF copy (per-query scale)

The span skeleton is the ACT exp stream over the causal TxT scores
(~139k free-dim cycles = ~116us minimum). v3 arranges everything so that
stream starts early and never starves:
  - emission order k0(+|k|^2 chain), q0, v(all tiles, PSUM->SBUF copy
    UNFOLDED so PSUM frees immediately), then k1,q1,k2,q2,k3,q3; the
    attention block runs at elevated priority from the q0 mark, so S-exp
    and AV for pair p start as soon as (q_p, k_p, v-fold_p) exist.
  - v fold is in-place in SBUF per (tile, pair) slice, emitted after that
    pair's f_col transposes; AV(p) only depends on its own pair's fold.
  - |k|^2 from k_pack: DVE square -> block-column-ones matmul -> [2,512]
    exps into f8 rows 32p (32-aligned partition bases) -> per-pair
    [32,128] PE transposes -> one strided DVE copy into f_colw.
  - all matmul operands bf16: FWL halves LDWEIGHTS, lower PE power keeps
    the HAM/SW clock throttle away (fp32r baseline sat at ~1.2 GHz).
  - attention S tiles are 512-col chunks with both heads of a pair packed
    [hh0 | hh1] in one [128,1024] PSUM tile -> one ACT exp per chunk.
  - causal mask multiplies on the otherwise-idle GPSIMD engine.
  - warmup matmuls on a zero tile cover the input-DMA window so the PE
    enters the kernel at K=8/8.

Host side: the two head-group cores of one batch are summed (the c_proj
row-parallel all-reduce) + b_proj.
"""

import math
import os
from contextlib import ExitStack

import numpy as np
import ml_dtypes

import concourse.bass as bass
import concourse.mybir as mybir
import concourse.tile as tile
from concourse.vector_clock import ScopedClock, VectorClock
from concourse.bass_utils import run_bass_kernel_spmd

F32 = mybir.dt.float32
BF16 = mybir.dt.bfloat16
AF = mybir.ActivationFunctionType
ALU = mybir.AluOpType

B, T, C, H = 4, 2048, 1024, 16
HD = C // H          # 64
HG = H // 2          # 8 heads per core
GC = HG * HD         # 512
NT = T // 128        # 16
NKC = C // 128       # 8
SCALE = -1.0 / (2.0 * math.sqrt(HD))   # -1/16
WU_MM = 48           # warmup matmuls to cover the input-DMA window

LAST_RESULTS = None
_last_in_maps = None


class _TC(tile.TileContext):
    """Tail barrier emitting one NOP per proc tick; this walrus build
    accepts only a single sync wait per instruction."""

    def _drain_and_barrier(self, tick_clock, wait_clock):
        gc = tick_clock.global_clock
        for proc in range(len(gc)):
            if gc[proc] <= 0:
                continue
            vc = VectorClock()
            vc.require_at_least(proc, gc[proc])
            nop_inst = self.nc.sync.nop(nofuse=True)
            wait_clock.add_sem_waits(nop_inst.ins, ScopedClock({None: vc}))
        self.nc.sync.drain()
        self.nc.all_engine_barrier()
        assert self.sems is not None
        popped = self.nc._tile_sem_poison_stack.pop()
        assert popped is self._sem_poison
        self.nc.clear_and_free_semaphores(list(self.sems.allocated().values()))
        self.nc.all_engine_barrier()


def _split_sync_waits(nc, keep=1):
    """Move excess per-instruction sem waits onto NOPs inserted just before,
    same engine stream (walrus here rejects >1 sync wait per instruction)."""
    for f in nc.m.functions:
        for bb in f.blocks:
            out = []
            changed = False
            for inst in bb.instructions:
                si = inst.sync_info
                waits = list(si.on_wait) if (si is not None and si.on_wait) else []
                if len(waits) > keep:
                    changed = True
                    for w in waits[:-keep]:
                        nop = mybir.InstNoOp(
                            name=f"I-wsplit-{nc.next_id()}", ins=[], outs=[]
                        )
                        nop.engine = inst.engine
                        nop.sync_info = mybir.SyncInfo(on_wait=[w], on_update=[])
                        out.append(nop)
                    ups = list(si.on_update) if si.on_update else []
                    inst.sync_info = mybir.SyncInfo(
                        on_wait=waits[-keep:], on_update=ups
                    )
                out.append(inst)
            if changed:
                bb.instructions = out


def _ap3(base, mid_stride, mid_n, inner_n):
    """[128, mid_n, inner_n] view of a 2-D AP with the given middle stride."""
    return bass.AP(
        base.tensor, base.offset,
        [list(base.ap[0]), [mid_stride, mid_n], [1, inner_n]],
    )


def _chunks(q0, q_hi):
    """512-grid-aligned [a0, a1) chunks covering [q0, q_hi)."""
    out = []
    a0 = q0
    while a0 < q_hi:
        a1 = min((a0 // 512 + 1) * 512, q_hi)
        out.append((a0, a1))
        a0 = a1
    return out


def _build_program():
    nc = bass.Bass(target_bir_lowering=False, trn_type="TRN2", debug=False)

    xT_d = nc.dram_tensor("xT", [C, T], BF16, kind="ExternalInput").ap()
    Wq_d = nc.dram_tensor("Wq", [C, GC], BF16, kind="ExternalInput").ap()
    Wk_d = nc.dram_tensor("Wk", [C, GC], BF16, kind="ExternalInput").ap()
    Wv_d = nc.dram_tensor("Wv", [C, GC], BF16, kind="ExternalInput").ap()
    bqc_d = nc.dram_tensor("bq_col", [128, 4], F32, kind="ExternalInput").ap()
    bkc_d = nc.dram_tensor("bk_col", [128, 4], F32, kind="ExternalInput").ap()
    bv_d = nc.dram_tensor("bv", [1, GC], BF16, kind="ExternalInput").ap()
    Wp_d = nc.dram_tensor("Wp", [GC, C], BF16, kind="ExternalInput").ap()
    mask_d = nc.dram_tensor("trimask", [128, 128], BF16, kind="ExternalInput").ap()
    id_d = nc.dram_tensor("ident", [128, 128], BF16, kind="ExternalInput").ap()
    out_d = nc.dram_tensor("out", [T, C], F32, kind="ExternalOutput").ap()

    with _TC(nc) as tc, ExitStack() as ctx:
        res = ctx.enter_context(tc.tile_pool(name="res", bufs=1))
        ps = ctx.enter_context(tc.tile_pool(name="ps", bufs=4, space="PSUM"))

        def big(nm):
            return ps.tile([128, 1024], F32, tag="big", name=nm)

        # ---- resident small tensors (input-critical DMAs only) ----
        mask = res.tile([128, 128], BF16, tag="mask")
        nc.sync.dma_start(mask[:], mask_d[:])
        ident = res.tile([128, 128], BF16, tag="ident")
        nc.sync.dma_start(ident[:], id_d[:])
        bqc = res.tile([128, 4], F32, tag="bqc")
        nc.sync.dma_start(bqc[:], bqc_d[:])
        bkc = res.tile([128, 4], F32, tag="bkc")
        nc.sync.dma_start(bkc[:], bkc_d[:])
        ones_r = res.tile([1, 128], BF16, tag="ones_r")
        nc.vector.memset(ones_r[:], 1.0)
        # block-diag 0.25 for |q~|^2/4 partition-reduce (q~ = -2q)
        ones_q = res.tile([128, 128], BF16, tag="ones_q")
        nc.vector.memset(ones_q[:], 0.0)
        nc.vector.memset(ones_q[0:64, 0:64], 0.25)
        nc.vector.memset(ones_q[64:128, 64:128], 0.25)
        # block-column ones for |k|^2 partition-reduce
        ones_k2 = res.tile([128, 2], BF16, tag="ones_k2")
        nc.vector.memset(ones_k2[:], 0.0)
        nc.vector.memset(ones_k2[0:64, 0:1], 1.0)
        nc.vector.memset(ones_k2[64:128, 1:2], 1.0)
        wu = res.tile([128, 512], BF16, tag="wu")
        nc.vector.memset(wu[:], 0.0)
        # f8[32p+hh, t] = exp(scale*|k|^2); f_colw col tt*128+32p+hh
        f8 = res.tile([128, T], BF16, tag="f8")
        nc.vector.memset(f8[:], 0.0)
        f_colw = res.tile([128, T], BF16, tag="f_colw")
        bv = res.tile([1, GC], BF16, tag="bv")
        wpp = ctx.enter_context(tc.tile_pool(name="wpp", bufs=1))
        wp = [wpp.tile([128, C], BF16, tag=f"wp{p}", name=f"wp{p}")
              for p in range(4)]

        # ---- resident big tensors ----
        xt_pool = ctx.enter_context(tc.tile_pool(name="xt", bufs=1))
        xT = []
        for kc in range(NKC):
            xt_t = xt_pool.tile([128, T], BF16, tag=f"x{kc}", name=f"xT{kc}")
            nc.sync.dma_start(xt_t[:], xT_d[kc * 128:(kc + 1) * 128, :])
            xT.append(xt_t)
        qk = ctx.enter_context(tc.tile_pool(name="qk", bufs=1))
        q_pack = [qk.tile([128, T], BF16, tag=f"q{p}", name=f"q_pack{p}")
                  for p in range(4)]
        k_pack = [qk.tile([128, T], BF16, tag=f"k{p}", name=f"k_pack{p}")
                  for p in range(4)]
        vt = ctx.enter_context(tc.tile_pool(name="vt", bufs=1))
        v_sb = [vt.tile([128, GC], BF16, tag=f"v{t}", name=f"v_sb{t}")
                for t in range(NT)]
        yp = ctx.enter_context(tc.tile_pool(name="yp", bufs=1))
        y_sb = [yp.tile([128, T], BF16, tag=f"y{p}", name=f"y_sb{p}")
                for p in range(4)]
        q2p = ctx.enter_context(tc.tile_pool(name="q2p", bufs=1))
        q2eh = [q2p.tile([128, T], BF16, tag=f"e{p}", name=f"q2e{p}")
                for p in range(4)]

        # ---- PE warmup while input DMAs land ----
        pwu = big("pwu")
        for i in range(WU_MM):
            nc.tensor.matmul(pwu[:, 0:512], wu[:, 0:128], wu[:, 0:512],
                             start=True, stop=True)

        with tc.tile_pool(name="wst", bufs=4) as wst, \
             tc.tile_pool(name="sqk", bufs=2) as sqk, \
             tc.tile_pool(name="wr", bufs=1) as wr:

            def emit_qk(p8):
                w_d = Wq_d if p8 < 4 else Wk_d
                bcol = bqc if p8 < 4 else bkc
                m0 = 128 * (p8 % 4)
                dst = q_pack[p8 % 4] if p8 < 4 else k_pack[p8 % 4]
                bigs = [big(f"psqk{p8}_{n2}") for n2 in range(2)]
                for kc in range(NKC):
                    wti = wst.tile([128, 128], BF16, tag="w", name=f"w{p8}_{kc}")
                    nc.sync.dma_start(
                        wti[:], w_d[kc * 128:(kc + 1) * 128, m0:m0 + 128])
                    for n in range(4):
                        nc.tensor.matmul(
                            bigs[n // 2][:, (n % 2) * 512:(n % 2) * 512 + 512],
                            wti[:], xT[kc][:, n * 512:(n + 1) * 512],
                            start=(kc == 0), stop=(kc == NKC - 1))
                for n2 in range(2):
                    nc.vector.tensor_scalar_add(
                        dst[:, n2 * 1024:(n2 + 1) * 1024], bigs[n2][:],
                        bcol[:, (p8 % 4):(p8 % 4) + 1])

            def emit_k2_chain(p):
                """|k|^2 -> exp -> f_colw cols tt*128+32p+{0,1} for pair p."""
                sq = sqk.tile([128, T], BF16, tag="sqk", name=f"sqk{p}")
                nc.vector.tensor_mul(sq[:], k_pack[p][:], k_pack[p][:])
                pk2 = big(f"pk2_{p}")
                for c in range(4):
                    bp = 32 * (c // 2)
                    co = 512 * (c % 2)
                    nc.tensor.matmul(
                        pk2[bp:bp + 2, co:co + 512], ones_k2[:, 0:2],
                        sq[:, c * 512:(c + 1) * 512],
                        start=True, stop=True, tile_position=(0, bp))
                for c in range(4):
                    bp = 32 * (c // 2)
                    co = 512 * (c % 2)
                    nc.scalar.activation(
                        f8[32 * p:32 * p + 2, c * 512:(c + 1) * 512],
                        pk2[bp:bp + 2, co:co + 512], AF.Exp, scale=SCALE)
                ptrp = big(f"ptr{p}")
                ptb = ptrp[:].bitcast(BF16)
                for tt in range(NT):
                    nc.tensor.transpose(
                        ptb[:, tt * 32:(tt + 1) * 32],
                        f8[32 * p:32 * p + 32, tt * 128:(tt + 1) * 128],
                        ident[32 * p:32 * p + 32, 32 * p:32 * p + 32],
                        tile_position=(32 * p, 0))
                src = bass.AP(ptb.tensor, ptb.offset,
                              [list(ptb.ap[0]), [32, NT], [1, 32]])
                fd = f_colw[:, 32 * p:]
                dst = bass.AP(fd.tensor, fd.offset,
                              [list(fd.ap[0]), [128, NT], [1, 32]])
                nc.vector.tensor_copy(dst, src)

            def emit_vfold(tt, p):
                """In-place v_sb[tt] *= exp(scale*|k|^2) for pair p's slice."""
                vs = v_sb[tt][:, p * 128:(p + 1) * 128]
                v3 = bass.AP(vs.tensor, vs.offset,
                             [list(vs.ap[0]), [HD, 2], [1, HD]])
                fb = f_colw[:, tt * 128 + 32 * p:]
                fb_b = bass.AP(fb.tensor, fb.offset,
                               [list(fb.ap[0]), [1, 2], [0, HD]])
                nc.vector.tensor_tensor(v3, v3, fb_b, op=ALU.mult)

            # pair 0 first: attention(p0) + v can start early
            emit_qk(4)
            emit_k2_chain(0)
            emit_qk(0)
            prio_mark = tc.cur_priority

            # ======== v-natural GEMM, unfolded copy, pair-0 folds ========
            nc.sync.dma_start(bv[:], bv_d[:])
            wv_r = [wr.tile([128, GC], BF16, tag=f"wr{kc}", name=f"wvr{kc}")
                    for kc in range(NKC)]
            for kc in range(NKC):
                nc.sync.dma_start(wv_r[kc][:], Wv_d[kc * 128:(kc + 1) * 128, :])
            for tt in range(NT):
                pv = big(f"pv{tt}")
                nc.tensor.matmul(pv[:, 0:GC], ones_r[0:1, 0:128], bv[0:1, :],
                                 start=True, stop=False)
                for kc in range(NKC):
                    nc.tensor.matmul(pv[:, 0:GC],
                                     xT[kc][:, tt * 128:tt * 128 + 128],
                                     wv_r[kc][:], start=False,
                                     stop=(kc == NKC - 1))
                nc.vector.tensor_copy(v_sb[tt][:], pv[:, 0:GC])
                emit_vfold(tt, 0)

            for p in range(4):
                nc.sync.dma_start(wp[p][:], Wp_d[p * 128:(p + 1) * 128, :])

            for p in (1, 2, 3):
                emit_qk(4 + p)
                emit_k2_chain(p)
                for tt in range(NT):
                    emit_vfold(tt, p)
                emit_qk(p)

        # ================= attention (elevated priority) =================
        with tc.tile_pool(name="sqq", bufs=2) as sqq, \
             tc.tile_pool(name="ssb", bufs=4) as ssb, \
             tc.tile_pool(name="osb", bufs=2) as osb:

            with tc.high_priority(offset=max(0, tc.cur_priority - prio_mark)):
                for half in range(2):
                    q_lo, q_hi = 1024 * half, 1024 * (half + 1)
                    for p in range(4):
                        # q2e for this (pair, half)
                        sq_q = sqq.tile([128, 1024], BF16, tag="sqq",
                                        name=f"sqq{p}_{half}")
                        nc.vector.tensor_mul(sq_q[:], q_pack[p][:, q_lo:q_hi],
                                             q_pack[p][:, q_lo:q_hi])
                        pq2 = big(f"pq2_{p}{half}")
                        for j in range(2):
                            nc.tensor.matmul(
                                pq2[:, j * 512:(j + 1) * 512], ones_q[:],
                                sq_q[:, j * 512:(j + 1) * 512],
                                start=True, stop=True)
                        nc.scalar.activation(q2eh[p][:, q_lo:q_hi], pq2[:],
                                             AF.Exp, scale=SCALE)

                        y_ps = big(f"yps{p}_{half}")
                        kt_last = 8 * half + 7
                        for kt in range(kt_last + 1):
                            q0 = max(128 * kt, q_lo)
                            for (a0, a1) in _chunks(q0, q_hi):
                                w = a1 - a0
                                s_ps = big(f"sps{p}_{half}_{kt}_{a0}")
                                for hh in range(2):
                                    nc.tensor.matmul(
                                        s_ps[:, hh * 512:hh * 512 + w],
                                        k_pack[p][hh * 64:hh * 64 + 64,
                                                  kt * 128:kt * 128 + 128],
                                        q_pack[p][hh * 64:hh * 64 + 64,
                                                  a0:a1],
                                        start=True, stop=True,
                                        tile_position=(hh * 64, 0))
                                s_sb = ssb.tile([128, 1024], BF16, tag="s",
                                                name=f"ssb{p}_{half}_{kt}_{a0}")
                                nc.scalar.activation(
                                    _ap3(s_sb[:], 512, 2, w),
                                    _ap3(s_ps[:], 512, 2, w),
                                    AF.Exp, scale=SCALE)
                                if a0 == 128 * kt and 128 * kt >= q_lo:
                                    mb = mask[:]
                                    nc.gpsimd.tensor_tensor(
                                        _ap3(s_sb[:], 512, 2, 128),
                                        _ap3(s_sb[:], 512, 2, 128),
                                        bass.AP(mb.tensor, mb.offset,
                                                [list(mb.ap[0]), [0, 2],
                                                 [1, 128]]),
                                        op=ALU.mult)
                                for hh in range(2):
                                    h = 2 * p + hh
                                    nc.tensor.matmul(
                                        y_ps[hh * 64:hh * 64 + 64,
                                             a0 - q_lo:a1 - q_lo],
                                        v_sb[kt][:, h * HD:h * HD + HD],
                                        s_sb[:, hh * 512:hh * 512 + w],
                                        start=(kt == 0), stop=(kt == kt_last),
                                        tile_position=(0, hh * 64))
                        nc.vector.tensor_tensor(
                            y_sb[p][:, q_lo:q_hi], y_ps[:],
                            q2eh[p][:, q_lo:q_hi], op=ALU.mult)

                    # ---- c_proj for this T-half ----
                    for tt in range(8 * half, 8 * half + 8):
                        po = big(f"po{tt}")
                        for p4 in range(4):
                            for n2 in range(2):
                                nc.tensor.matmul(
                                    po[:, n2 * 512:(n2 + 1) * 512],
                                    y_sb[p4][:, tt * 128:(tt + 1) * 128],
                                    wp[p4][:, n2 * 512:(n2 + 1) * 512],
                                    start=(p4 == 0), stop=(p4 == 3))
                        o_sb = osb.tile([128, C], F32, tag="o", name=f"osb{tt}")
                        nc.vector.tensor_copy(o_sb[:], po[:])
                        nc.sync.dma_start(out_d[tt * 128:(tt + 1) * 128, :],
                                          o_sb[:])

    _split_sync_waits(nc)
    return nc


_NC_CACHE = None


def _get_program():
    global _NC_CACHE
    if _NC_CACHE is None:
        _NC_CACHE = _build_program()
    return _NC_CACHE


def kernel(x, W_attn, b_attn, W_proj, b_proj, n_head):
    global LAST_RESULTS, _last_in_maps
    assert int(n_head) == H
    x = np.asarray(x, dtype=np.float32)
    W_attn = np.asarray(W_attn, dtype=np.float32)
    b_attn = np.asarray(b_attn, dtype=np.float32)
    W_proj = np.asarray(W_proj, dtype=np.float32)
    b_proj = np.asarray(b_proj, dtype=np.float32)

    bf = ml_dtypes.bfloat16
    mask = np.triu(np.ones((128, 128), np.float32)).astype(bf)
    ident = np.eye(128, dtype=np.float32).astype(bf)

    in_maps = []
    for c in range(8):
        b = c // 2
        g = c % 2
        cols = slice(g * GC, (g + 1) * GC)
        bq = -2.0 * b_attn[0 * C:1 * C][cols]
        bk = b_attn[1 * C:2 * C][cols]
        in_maps.append({
            "xT": np.ascontiguousarray(x[b].T).astype(bf),
            "Wq": np.ascontiguousarray(
                -2.0 * W_attn[:, 0 * C:1 * C][:, cols]).astype(bf),
            "Wk": np.ascontiguousarray(
                W_attn[:, 1 * C:2 * C][:, cols]).astype(bf),
            "Wv": np.ascontiguousarray(
                W_attn[:, 2 * C:3 * C][:, cols]).astype(bf),
            "bq_col": np.ascontiguousarray(bq.reshape(4, 128).T),
            "bk_col": np.ascontiguousarray(bk.reshape(4, 128).T),
            "bv": b_attn[2 * C:3 * C][cols].reshape(1, GC).astype(bf),
            "Wp": np.ascontiguousarray(
                W_proj[g * GC:(g + 1) * GC, :]).astype(bf),
            "trimask": mask,
            "ident": ident,
        })

    _last_in_maps = in_maps
    nc = _get_program()
    LAST_RESULTS = run_bass_kernel_spmd(nc, in_maps, core_ids=list(range(8)))

    out = np.empty((B, T, C), np.float32)
    for b in range(B):
        out[b] = (LAST_RESULTS.results[2 * b]["out"]
                  + LAST_RESULTS.results[2 * b + 1]["out"] + b_proj)
    return out
